# revision 1
# baseline (speedup 1.0000x reference)
# Trainium2 Bass kernel for nn_MHAttentionMap (DETR-style attention map).
#
# Reference computation:
#   qp = q @ q_w.T + q_b                       [b, Q, 256]
#   kp = 1x1conv(k, k_w) + k_b                 [b, 256, H, W]
#   scores[b,q,n,s] = (qh*NORM) . kh           [b, Q, 8, H*W]
#   scores[mask] = -inf ; softmax over flattened (n, H, W) per (b, q)
#
# Sharding: 8 cores = (batch 0..3) x (query half 0..1); 150 queries/core.
# The softmax axis (heads x spatial) lives entirely on one core, so no
# collectives are needed.
#
# Per-core device program (identical on all cores, different data):
#   - qproj on PE -> qpT group tiles (fp16), NORM_FACT folded in
#   - kproj on PE (K=256) -> kp tiles [97/97/65, 10000] fp16 grouped as
#     heads (0-2), (3-5), (6-7); last row of each = mask bias row
#     (0 / -30000, fp16, marshaled on host)
#   - scores: block-diagonal matmuls packing (heads-in-group x query-block)
#     into M<=126 with a ones-row in lhsT so the mask bias adds inside the
#     matmul (K = 32*hg + 1)
#   - exp on ACT directly from PSUM into per-pass fp16 buffers with
#     accum_out partial row sums; fold/unfold 0/1 matmuls (host constants)
#     reduce per-(head,query) sums into per-query totals and broadcast the
#     reciprocal back to the pass layout
#   - in-place DVE normalize, DMA out
#
# Two query rounds of 75 keep the fp16 exp buffers inside SBUF.

import numpy as np

import concourse.bacc as bacc
import concourse.bass as bass
import concourse.mybir as mybir
import concourse.tile as tile
from concourse.bass_utils import run_bass_kernel_spmd

QUERY_DIM = 256
HIDDEN = 256
NH = 8
HD = HIDDEN // NH  # 32
NORM_FACT = float(HIDDEN / NH) ** (-0.5)

B = 4
Q = 300
H = 100
W = 100
S = H * W  # 10000
NCORES = 8
QS = Q // 2  # 150 queries per core

# head groups: (#heads, first head)
HGROUPS = [(3, 0), (3, 3), (2, 6)]
# query rounds of 75, each split into blocks of (42, 33)
ROUND_Q = 75
QBLOCKS = [(0, 42), (42, 33)]

MASK_NEG = -30000.0

# fp16 output halves the dominant HBM write; verified <2e-3 rel err.
OUT_DTYPE = mybir.dt.float16

F32 = mybir.dt.float32
F16 = mybir.dt.float16


def _chunks(total, size):
    out = []
    off = 0
    while off < total:
        out.append((off, min(size, total - off)))
        off += size
    return out


def _fold_consts():
    # fold[qs*r + j, j] = 1 folds 3 stacked per-head rows into per-query;
    # unfold is its transpose (broadcast back to pass layout).
    consts = {}
    for qs in (42, 33):
        fold = np.zeros((3 * qs, qs), np.float32)
        for r in range(3):
            fold[qs * r + np.arange(qs), np.arange(qs)] = 1.0
        consts[f"fold{qs}"] = fold
        consts[f"unfold{qs}"] = np.ascontiguousarray(fold.T)
    return consts


def _emit(nc, tc, ctx, d, use_qbias, use_kbias):
    """Emit the per-core program. d: dict of DRAM tensor handles."""
    consts = ctx.enter_context(tc.tile_pool(name="consts", bufs=1))
    persist = ctx.enter_context(tc.tile_pool(name="persist", bufs=1))
    work = ctx.enter_context(tc.tile_pool(name="work", bufs=3))
    small = ctx.enter_context(tc.tile_pool(name="small", bufs=4))
    psum = ctx.enter_context(tc.tile_pool(name="psum", bufs=2, space="PSUM"))

    # ---- load constants ----
    qwT = []
    kwT = []
    for kb in range(2):
        t = consts.tile([128, 256], F32, tag=f"qwT{kb}", name=f"qwT{kb}")
        nc.sync.dma_start(out=t, in_=d["q_wT"][kb * 128:(kb + 1) * 128, :])
        qwT.append(t)
        t2 = consts.tile([128, 256], F32, tag=f"kwT{kb}", name=f"kwT{kb}")
        nc.sync.dma_start(out=t2, in_=d["k_wT"][kb * 128:(kb + 1) * 128, :])
        kwT.append(t2)
    qT = []
    for kb in range(2):
        t = consts.tile([128, QS], F32, tag=f"qT{kb}", name=f"qT{kb}")
        nc.sync.dma_start(out=t, in_=d["qT"][kb * 128:(kb + 1) * 128, :])
        qT.append(t)
    foldc = {}
    unfoldc = {}
    for qs in (42, 33):
        f = consts.tile([3 * qs, qs], F32, tag=f"fold{qs}", name=f"fold{qs}")
        nc.sync.dma_start(out=f, in_=d[f"fold{qs}"][:, :])
        foldc[qs] = f
        u = consts.tile([qs, 3 * qs], F32, tag=f"unfold{qs}", name=f"unfold{qs}")
        nc.sync.dma_start(out=u, in_=d[f"unfold{qs}"][:, :])
        unfoldc[qs] = u
    qbias_t = []
    kbias_t = []
    if use_qbias or use_kbias:
        for g, (hg, h0) in enumerate(HGROUPS):
            mg = 32 * hg
            if use_qbias:
                t = consts.tile([mg, 1], F32, tag=f"qb{g}", name=f"qb{g}")
                nc.sync.dma_start(out=t, in_=d[f"qbias{g}"][:, :])
                qbias_t.append(t)
            if use_kbias:
                t = consts.tile([mg, 1], F32, tag=f"kb{g}", name=f"kb{g}")
                nc.sync.dma_start(out=t, in_=d[f"kbias{g}"][:, :])
                kbias_t.append(t)

    # ---- qproj: qpT_g[g] [32*hg, 150] fp16 = (q_w @ q.T + q_b) * NORM ----
    qpT = []
    for g, (hg, h0) in enumerate(HGROUPS):
        mg = 32 * hg
        moff = 32 * h0
        ps = psum.tile([mg, QS], F32, tag="ps", name=f"qproj_ps{g}")
        for kb in range(2):
            nc.tensor.matmul(
                ps[0:mg, 0:QS],
                qwT[kb][:, moff:moff + mg],
                qT[kb][:, 0:QS],
                start=(kb == 0),
                stop=(kb == 1),
            )
        t = persist.tile([mg, QS], F16, tag=f"qpT{g}", name=f"qpT{g}")
        bias = qbias_t[g][0:mg, 0:1] if use_qbias else 0.0
        nc.scalar.activation(
            t[0:mg, 0:QS], ps[0:mg, 0:QS],
            mybir.ActivationFunctionType.Identity,
            bias=bias, scale=NORM_FACT,
        )
        qpT.append(t)

    # ---- block-diagonal lhsT staging tiles (both rounds) ----
    # stg[(r, g, qb)]: [K_g, M_p] fp16, K_g = 32*hg + 1 (ones row last),
    # block r' at rows 32r'..32r'+32, cols r'*qs..(r'+1)*qs.
    stg = {}
    for r in range(2):
        for g, (hg, h0) in enumerate(HGROUPS):
            kg = 32 * hg + 1
            for qb, (q0, qs) in enumerate(QBLOCKS):
                mp = hg * qs
                t = persist.tile([kg, 126], F16, tag=f"stg_{r}_{g}_{qb}",
                                 name=f"stg_{r}_{g}_{qb}")
                nc.vector.memset(t, 0.0)
                qa = r * ROUND_Q + q0
                for rr in range(hg):
                    nc.vector.tensor_copy(
                        t[32 * rr:32 * rr + 32, rr * qs:(rr + 1) * qs],
                        qpT[g][32 * rr:32 * rr + 32, qa:qa + qs],
                    )
                nc.vector.memset(t[kg - 1:kg, 0:mp], 1.0)
                stg[(r, g, qb)] = t

    # ---- kproj: kp[g] [32*hg + 1, S] fp16, mask row last ----
    kp = []
    for g, (hg, h0) in enumerate(HGROUPS):
        kg = 32 * hg + 1
        t = persist.tile([kg, S], F16, tag=f"kp{g}", name=f"kp{g}")
        nc.sync.dma_start(out=t[kg - 1:kg, :], in_=d["maskb"][0:1, :])
        kp.append(t)

    for c0, cw in _chunks(S, 1024):
        kin = []
        for kb in range(2):
            t = work.tile([128, 1024], F32, tag=f"kin{kb}", bufs=2,
                          name=f"kin{kb}_{c0}")
            nc.sync.dma_start(out=t[:, 0:cw],
                              in_=d["k"][kb * 128:(kb + 1) * 128, c0:c0 + cw])
            kin.append(t)
        for g, (hg, h0) in enumerate(HGROUPS):
            mg = 32 * hg
            moff = 32 * h0
            ps = psum.tile([mg, 1024], F32, tag="ps", name=f"kproj_ps{g}_{c0}")
            for js, nw in _chunks(cw, 512):
                for kb in range(2):
                    nc.tensor.matmul(
                        ps[0:mg, js:js + nw],
                        kwT[kb][:, moff:moff + mg],
                        kin[kb][:, js:js + nw],
                        start=(kb == 0),
                        stop=(kb == 1),
                    )
            if use_kbias:
                nc.vector.tensor_scalar_add(
                    kp[g][0:mg, c0:c0 + cw], ps[0:mg, 0:cw],
                    kbias_t[g][0:mg, 0:1],
                )
            else:
                nc.vector.tensor_copy(kp[g][0:mg, c0:c0 + cw], ps[0:mg, 0:cw])

    # ---- rounds: scores -> exp(+accum) -> sums -> normalize -> out ----
    for r in range(2):
        expb = {}
        sums = {}
        for qb, (q0, qs) in enumerate(QBLOCKS):
            t = small.tile([126, 3], F32, tag=f"sums_{r}_{qb}", bufs=1,
                           name=f"sums_{r}_{qb}")
            nc.vector.memset(t, 0.0)
            sums[qb] = t

        for qb, (q0, qs) in enumerate(QBLOCKS):
            for g, (hg, h0) in enumerate(HGROUPS):
                kg = 32 * hg + 1
                mp = hg * qs
                eb = work.tile([126, S], F16, tag=f"expb_{g}_{qb}", bufs=1,
                               name=f"expb_{r}_{g}_{qb}")
                expb[(g, qb)] = eb
                parts = small.tile([126, 5], F32, tag="parts", bufs=3,
                                   name=f"parts_{r}_{g}_{qb}")
                lhs = stg[(r, g, qb)]
                for ci, (c0, cw) in enumerate(_chunks(S, 2048)):
                    ps = psum.tile([126, 2048], F32, tag="ps",
                                   name=f"sc_ps_{r}_{g}_{qb}_{c0}")
                    for js, nw in _chunks(cw, 512):
                        nc.tensor.matmul(
                            ps[0:mp, js:js + nw],
                            lhs[0:kg, 0:mp],
                            kp[g][0:kg, c0 + js:c0 + js + nw],
                            start=True, stop=True,
                        )
                    nc.scalar.activation(
                        eb[0:mp, c0:c0 + cw], ps[0:mp, 0:cw],
                        mybir.ActivationFunctionType.Exp,
                        accum_out=parts[0:mp, ci:ci + 1],
                    )
                nc.vector.tensor_reduce(
                    sums[qb][0:mp, g:g + 1], parts[0:mp, 0:5],
                    axis=mybir.AxisListType.X, op=mybir.AluOpType.add,
                )

        # per-query totals -> reciprocal -> broadcast to pass layout
        recP = {}
        for qb, (q0, qs) in enumerate(QBLOCKS):
            fps = psum.tile([qs, 3], F32, tag="ps", name=f"fold_ps_{r}_{qb}")
            nc.tensor.matmul(fps[0:qs, 0:3], foldc[qs][0:3 * qs, 0:qs],
                             sums[qb][0:3 * qs, 0:3], start=True, stop=True)
            tot = small.tile([qs, 1], F32, tag="tot", name=f"tot_{r}_{qb}")
            nc.vector.tensor_reduce(tot[0:qs, 0:1], fps[0:qs, 0:3],
                                    axis=mybir.AxisListType.X,
                                    op=mybir.AluOpType.add)
            rec = small.tile([qs, 1], F32, tag="rec", name=f"rec_{r}_{qb}")
            nc.vector.reciprocal(rec[0:qs, 0:1], tot[0:qs, 0:1])
            ups = psum.tile([3 * qs, 1], F32, tag="ps", name=f"unf_ps_{r}_{qb}")
            nc.tensor.matmul(ups[0:3 * qs, 0:1], unfoldc[qs][0:qs, 0:3 * qs],
                             rec[0:qs, 0:1], start=True, stop=True)
            rp = small.tile([126, 1], F32, tag=f"recP{qb}", bufs=2,
                            name=f"recP_{r}_{qb}")
            nc.vector.tensor_copy(rp[0:3 * qs, 0:1], ups[0:3 * qs, 0:1])
            recP[qb] = rp

        # normalize in place and write out
        out_r = d["out"][:].rearrange("q (h s) -> h q s", h=NH)
        for qb, (q0, qs) in enumerate(QBLOCKS):
            for g, (hg, h0) in enumerate(HGROUPS):
                mp = hg * qs
                eb = expb[(g, qb)]
                nc.vector.tensor_scalar_mul(
                    eb[0:mp, 0:S], eb[0:mp, 0:S], recP[qb][0:mp, 0:1],
                )
                qa = r * ROUND_Q + q0
                nc.sync.dma_start(
                    out=out_r[h0:h0 + hg, qa:qa + qs, :],
                    in_=eb[0:mp, 0:S],
                )


_CACHED = {}


def _build(use_qbias, use_kbias):
    key = (use_qbias, use_kbias)
    if key in _CACHED:
        return _CACHED[key]
    nc = bacc.Bacc("TRN2", target_bir_lowering=False, debug=False)
    d = {}
    d["qT"] = nc.dram_tensor("qT", [256, QS], F32, kind="ExternalInput")
    d["k"] = nc.dram_tensor("k", [256, S], F32, kind="ExternalInput")
    d["maskb"] = nc.dram_tensor("maskb", [1, S], F16, kind="ExternalInput")
    d["q_wT"] = nc.dram_tensor("q_wT", [256, 256], F32, kind="ExternalInput")
    d["k_wT"] = nc.dram_tensor("k_wT", [256, 256], F32, kind="ExternalInput")
    for qs in (42, 33):
        d[f"fold{qs}"] = nc.dram_tensor(f"fold{qs}", [3 * qs, qs], F32,
                                        kind="ExternalInput")
        d[f"unfold{qs}"] = nc.dram_tensor(f"unfold{qs}", [qs, 3 * qs], F32,
                                          kind="ExternalInput")
    if use_qbias:
        for g, (hg, h0) in enumerate(HGROUPS):
            d[f"qbias{g}"] = nc.dram_tensor(f"qbias{g}", [32 * hg, 1], F32,
                                            kind="ExternalInput")
    if use_kbias:
        for g, (hg, h0) in enumerate(HGROUPS):
            d[f"kbias{g}"] = nc.dram_tensor(f"kbias{g}", [32 * hg, 1], F32,
                                            kind="ExternalInput")
    d["out"] = nc.dram_tensor("out", [QS, NH * S], OUT_DTYPE,
                              kind="ExternalOutput")
    from contextlib import ExitStack
    with tile.TileContext(nc) as tc:
        with ExitStack() as ctx:
            _emit(nc, tc, ctx, d, use_qbias, use_kbias)
    nc.compile()
    _CACHED[key] = nc
    return nc


def make_in_maps(q, k, mask, q_w, q_b, k_w, k_b):
    use_qbias = bool(np.any(q_b != 0))
    use_kbias = bool(np.any(k_b != 0))
    shared = {
        "q_wT": np.ascontiguousarray(q_w.T.astype(np.float32)),
        "k_wT": np.ascontiguousarray(k_w.T.astype(np.float32)),
    }
    for fname, arr in _fold_consts().items():
        shared[fname] = arr
    if use_qbias:
        qb_scaled = (q_b.astype(np.float32) * NORM_FACT).reshape(256, 1)
        for g, (hg, h0) in enumerate(HGROUPS):
            shared[f"qbias{g}"] = np.ascontiguousarray(
                qb_scaled[32 * h0:32 * h0 + 32 * hg])
    if use_kbias:
        kb_col = k_b.astype(np.float32).reshape(256, 1)
        for g, (hg, h0) in enumerate(HGROUPS):
            shared[f"kbias{g}"] = np.ascontiguousarray(
                kb_col[32 * h0:32 * h0 + 32 * hg])

    in_maps = []
    for c in range(NCORES):
        b, qh = c // 2, c % 2
        m = dict(shared)
        m["qT"] = np.ascontiguousarray(
            q[b, qh * QS:(qh + 1) * QS, :].T.astype(np.float32))
        m["k"] = np.ascontiguousarray(
            k[b].reshape(256, S).astype(np.float32))
        m["maskb"] = np.where(mask[b].reshape(1, S), np.float16(MASK_NEG),
                              np.float16(0.0))
        in_maps.append(m)
    return in_maps, use_qbias, use_kbias


def assemble(outs):
    """outs: list of 8 per-core [QS, NH*S] arrays -> [B, Q, NH, H, W] f32."""
    full = np.empty((B, Q, NH, H, W), np.float32)
    for c in range(NCORES):
        b, qh = c // 2, c % 2
        full[b, qh * QS:(qh + 1) * QS] = (
            outs[c].astype(np.float32).reshape(QS, NH, H, W))
    return full


def kernel(q, k, mask, q_w, q_b, k_w, k_b, _trace=False):
    in_maps, use_qbias, use_kbias = make_in_maps(q, k, mask, q_w, q_b, k_w, k_b)
    nc = _build(use_qbias, use_kbias)
    res = run_bass_kernel_spmd(nc, in_maps, core_ids=list(range(NCORES)),
                               trace=_trace)
    out = assemble([r["out"] for r in res.results])
    if _trace:
        return out, res
    return out



# revision 2
# speedup vs baseline: 41956.5639x; 41956.5639x over previous
# Trainium2 Bass kernel for nn_MHAttentionMap (DETR-style attention map), v2.
#
# Reference computation:
#   qp = q @ q_w.T + q_b                       [b, Q, 256]
#   kp = 1x1conv(k, k_w) + k_b                 [b, 256, H, W]
#   scores[b,q,n,s] = (qh*NORM) . kh           [b, Q, 8, H*W]
#   scores[mask] = -inf ; softmax over flattened (n, H, W) per (b, q)
#
# Sharding: 8 cores = (batch 0..3) x (query half 0..1); 150 queries/core.
# The softmax axis (heads x spatial) lives entirely on one core, so no
# collectives are needed.
#
# v4 changes vs v2/v3 (300us / 295us device time):
#   - fin(i) (fold chain + normalize + out DMA) is emitted between
#     scores(i+1, g0) and scores(i+1, g1): in v2/v3 the fold matmul sat
#     behind ALL of scores(i+1) in PE program order, and those matmuls
#     are paced by PSUM rotation against the exp stream — so every
#     block's flush started ~20us late and starved the exp stream of
#     its (bufs=2) buffers. ACT is the critical engine (~124us exp
#     stream); this ordering keeps it saturated.
#   - k input DMAs alternate sync/gpsimd (both idle early); the scalar
#     queue carries no DMAs until the final flush so exp is never
#     blocked behind a transfer.
#   - column-split output DMAs + final block flushing over three queues
#     (sync/gpsimd/scalar) to shrink the drain tail.
#   - qproj scale on DVE, not ACT.
#
# Per-core device program (identical on all cores, different data):
#   - qproj on PE -> qpT group tiles (fp16), NORM_FACT folded in
#   - kproj on PE (K=256, fp16) -> kp tiles [97/97/65, 10000] fp16 grouped
#     as heads (0-2), (3-5), (6-7); last row of each = mask bias row
#     (0 / -30000, fp16, marshaled on host)
#   - scores: block-diagonal matmuls packing (heads-in-group x query-block)
#     into M<=126 with a ones-row in lhsT so the mask bias adds inside the
#     matmul (K = 32*hg + 1)
#   - exp on ACT directly from PSUM into fp16 block buffers with
#     accum_out partial row sums; fold/unfold 0/1 matmuls (host constants)
#     reduce per-(head,query) sums into per-query totals and broadcast the
#     reciprocal back to the block layout
#   - in-place DVE normalize, per-head DMA out on alternating queues

import numpy as np

import concourse.bacc as bacc
import concourse.bass as bass
import concourse.mybir as mybir
import concourse.tile as tile
from concourse.bass_utils import run_bass_kernel_spmd

QUERY_DIM = 256
HIDDEN = 256
NH = 8
HD = HIDDEN // NH  # 32
NORM_FACT = float(HIDDEN / NH) ** (-0.5)

B = 4
Q = 300
H = 100
W = 100
S = H * W  # 10000
NCORES = 8
QS = Q // 2  # 150 queries per core

# head groups: (#heads, first head)
HGROUPS = [(3, 0), (3, 3), (2, 6)]
# query blocks: (offset, size)
QBLOCKS = [(0, 42), (42, 42), (84, 42), (126, 24)]
QSIZES = (42, 24)

MASK_NEG = -30000.0

# fp16 output halves the dominant HBM write; verified <2e-3 rel err.
OUT_DTYPE = mybir.dt.float16

F32 = mybir.dt.float32
F16 = mybir.dt.float16


def _chunks(total, size):
    out = []
    off = 0
    while off < total:
        out.append((off, min(size, total - off)))
        off += size
    return out


_FOLD_CONSTS = None


def _fold_consts():
    # fold[qs*r + j, j] = 1 folds 3 stacked per-head rows into per-query;
    # unfold is its transpose (broadcast back to block layout).
    global _FOLD_CONSTS
    if _FOLD_CONSTS is None:
        consts = {}
        for qs in QSIZES:
            fold = np.zeros((3 * qs, qs), np.float32)
            for r in range(3):
                fold[qs * r + np.arange(qs), np.arange(qs)] = 1.0
            consts[f"fold{qs}"] = fold
            consts[f"unfold{qs}"] = np.ascontiguousarray(fold.T)
        _FOLD_CONSTS = consts
    return _FOLD_CONSTS


def _emit(nc, tc, ctx, d, use_qbias, use_kbias):
    """Emit the per-core program. d: dict of DRAM tensor handles."""
    consts = ctx.enter_context(tc.tile_pool(name="consts", bufs=1))
    persist = ctx.enter_context(tc.tile_pool(name="persist", bufs=1))
    work = ctx.enter_context(tc.tile_pool(name="work", bufs=3))
    small = ctx.enter_context(tc.tile_pool(name="small", bufs=4))
    psum = ctx.enter_context(tc.tile_pool(name="psum", bufs=2, space="PSUM"))

    # ---- load constants (sync queue; small) ----
    # qproj inputs first so the PE can start immediately
    qwT = []
    qT = []
    for kb in range(2):
        t = consts.tile([128, 256], F16, tag=f"qwT{kb}", name=f"qwT{kb}")
        nc.sync.dma_start(out=t, in_=d["q_wT"][kb * 128:(kb + 1) * 128, :])
        qwT.append(t)
        t2 = consts.tile([128, QS], F16, tag=f"qT{kb}", name=f"qT{kb}")
        nc.sync.dma_start(out=t2, in_=d["qT"][kb * 128:(kb + 1) * 128, :])
        qT.append(t2)
    kwT = []
    for kb in range(2):
        t = consts.tile([128, 256], F16, tag=f"kwT{kb}", name=f"kwT{kb}")
        nc.sync.dma_start(out=t, in_=d["k_wT"][kb * 128:(kb + 1) * 128, :])
        kwT.append(t)
    foldc = {}
    unfoldc = {}
    for qs in QSIZES:
        f = consts.tile([3 * qs, qs], F32, tag=f"fold{qs}", name=f"fold{qs}")
        nc.sync.dma_start(out=f, in_=d[f"fold{qs}"][:, :])
        foldc[qs] = f
        u = consts.tile([qs, 3 * qs], F32, tag=f"unfold{qs}", name=f"unfold{qs}")
        nc.sync.dma_start(out=u, in_=d[f"unfold{qs}"][:, :])
        unfoldc[qs] = u
    qbias_t = []
    kbias_t = []
    if use_qbias or use_kbias:
        for g, (hg, h0) in enumerate(HGROUPS):
            mg = 32 * hg
            if use_qbias:
                t = consts.tile([mg, 1], F32, tag=f"qb{g}", name=f"qb{g}")
                nc.sync.dma_start(out=t, in_=d[f"qbias{g}"][:, :])
                qbias_t.append(t)
            if use_kbias:
                t = consts.tile([mg, 1], F32, tag=f"kb{g}", name=f"kb{g}")
                nc.sync.dma_start(out=t, in_=d[f"kbias{g}"][:, :])
                kbias_t.append(t)

    # ---- qproj: qpT_g[g] [32*hg, 150] fp16 = (q_w @ q.T + q_b) * NORM ----
    # scale on DVE, not ACT: the ACT engine is the critical exp stream.
    qpT = []
    for g, (hg, h0) in enumerate(HGROUPS):
        mg = 32 * hg
        moff = 32 * h0
        ps = psum.tile([126, 2048], F32, tag="ps", name=f"qproj_ps{g}")
        for kb in range(2):
            nc.tensor.matmul(
                ps[0:mg, 0:QS],
                qwT[kb][:, moff:moff + mg],
                qT[kb][:, 0:QS],
                start=(kb == 0),
                stop=(kb == 1),
            )
        t = persist.tile([mg, QS], F16, tag=f"qpT{g}", name=f"qpT{g}")
        nc.vector.tensor_scalar_mul(t[0:mg, 0:QS], ps[0:mg, 0:QS], NORM_FACT)
        if use_qbias:
            # host ships q_b * NORM_FACT, so add after the scale
            nc.vector.tensor_scalar_add(
                t[0:mg, 0:QS], t[0:mg, 0:QS], qbias_t[g][0:mg, 0:1])
        qpT.append(t)

    # ---- block-diagonal lhsT staging tiles (all query blocks) ----
    # stg[(qb, g)]: [K_g, M_p] fp16, K_g = 32*hg + 1 (ones row last),
    # block r at rows 32r..32r+32, cols r*qs..(r+1)*qs.
    stg = {}
    for qb, (q0, qs) in enumerate(QBLOCKS):
        for g, (hg, h0) in enumerate(HGROUPS):
            kg = 32 * hg + 1
            mp = hg * qs
            t = persist.tile([kg, 126], F16, tag=f"stg_{qb}_{g}",
                             name=f"stg_{qb}_{g}")
            nc.vector.memset(t, 0.0)
            for rr in range(hg):
                nc.vector.tensor_copy(
                    t[32 * rr:32 * rr + 32, rr * qs:(rr + 1) * qs],
                    qpT[g][32 * rr:32 * rr + 32, q0:q0 + qs],
                )
            nc.vector.memset(t[kg - 1:kg, 0:mp], 1.0)
            stg[(qb, g)] = t

    # ---- kproj: kp[g] [32*hg + 1, S] fp16, mask row last ----
    kp = []
    for g, (hg, h0) in enumerate(HGROUPS):
        kg = 32 * hg + 1
        t = persist.tile([kg, S], F16, tag=f"kp{g}", name=f"kp{g}")
        nc.sync.dma_start(out=t[kg - 1:kg, :], in_=d["maskb"][0:1, :])
        kp.append(t)

    # k input chunks stream through small double-buffered tiles on the
    # sync+gpsimd queues (both idle early; scalar stays clean for exp).
    def kproj_chunk(ci, c0, cw):
        kin = []
        for kb in range(2):
            t = work.tile([128, 2048], F16, tag=f"kin{kb}", bufs=2,
                          name=f"kin{kb}_{c0}")
            eng = nc.sync if (ci + kb) % 2 == 0 else nc.gpsimd
            eng.dma_start(out=t[:, 0:cw],
                          in_=d["k"][kb * 128:(kb + 1) * 128, c0:c0 + cw])
            kin.append(t)
        for g, (hg, h0) in enumerate(HGROUPS):
            mg = 32 * hg
            moff = 32 * h0
            ps = psum.tile([126, 2048], F32, tag="ps",
                           name=f"kproj_ps{g}_{c0}")
            for js, nw in _chunks(cw, 512):
                for kb in range(2):
                    nc.tensor.matmul(
                        ps[0:mg, js:js + nw],
                        kwT[kb][:, moff:moff + mg],
                        kin[kb][:, js:js + nw],
                        start=(kb == 0),
                        stop=(kb == 1),
                    )
            if use_kbias:
                nc.vector.tensor_scalar_add(
                    kp[g][0:mg, c0:c0 + cw], ps[0:mg, 0:cw],
                    kbias_t[g][0:mg, 0:1],
                )
            else:
                nc.vector.tensor_copy(kp[g][0:mg, c0:c0 + cw],
                                      ps[0:mg, 0:cw])

    # ---- streaming query blocks: scores -> exp(+accum) -> (deferred
    # fold -> normalize -> out) so block i+1 compute overlaps block i
    # normalize + DMA out. expb rotates 2 buffers per group.
    sums = {}
    expb = {}
    out_r = d["out"][:].rearrange("q (h s) -> h q s", h=NH)
    dma_idx = 0

    def emit_sums_tile(qb):
        t = small.tile([126, 3], F32, tag="sums", bufs=4, name=f"sums_{qb}")
        nc.vector.memset(t, 0.0)
        sums[qb] = t

    def alloc_scores(qb, g):
        eb = work.tile([126, S], F16, tag=f"expb_{g}", bufs=2,
                       name=f"expb_{qb}_{g}")
        expb[(qb, g)] = eb
        parts = small.tile([126, 5], F32, tag="parts", bufs=3,
                           name=f"parts_{qb}_{g}")
        return eb, parts

    def emit_score_chunk(qb, g, ci, c0, cw, eb, parts):
        q0, qs = QBLOCKS[qb]
        hg, h0 = HGROUPS[g]
        kg = 32 * hg + 1
        mp = hg * qs
        lhs = stg[(qb, g)]
        ps = psum.tile([126, 2048], F32, tag="ps",
                       name=f"sc_ps_{qb}_{g}_{c0}")
        for js, nw in _chunks(cw, 512):
            nc.tensor.matmul(
                ps[0:mp, js:js + nw],
                lhs[0:kg, 0:mp],
                kp[g][0:kg, c0 + js:c0 + js + nw],
                start=True, stop=True,
            )
        nc.scalar.activation(
            eb[0:mp, c0:c0 + cw], ps[0:mp, 0:cw],
            mybir.ActivationFunctionType.Exp,
            accum_out=parts[0:mp, ci:ci + 1],
        )

    def emit_score_reduce(qb, g, parts, nchunks):
        q0, qs = QBLOCKS[qb]
        hg, h0 = HGROUPS[g]
        mp = hg * qs
        nc.vector.tensor_reduce(
            sums[qb][0:mp, g:g + 1], parts[0:mp, 0:nchunks],
            axis=mybir.AxisListType.X, op=mybir.AluOpType.add,
        )

    def emit_scores(qb, groups):
        for g in groups:
            eb, parts = alloc_scores(qb, g)
            nchunks = _chunks(S, 2048)
            for ci, (c0, cw) in enumerate(nchunks):
                emit_score_chunk(qb, g, ci, c0, cw, eb, parts)
            emit_score_reduce(qb, g, parts, len(nchunks))

    def emit_finish(qb, final=False):
        nonlocal dma_idx
        q0, qs = QBLOCKS[qb]
        # per-query totals -> reciprocal -> broadcast to block layout.
        # column-sum first so the PE fold feeds straight into reciprocal.
        csum = small.tile([126, 1], F32, tag="csum", bufs=2,
                          name=f"csum_{qb}")
        nc.vector.tensor_reduce(csum[0:3 * qs, 0:1], sums[qb][0:3 * qs, 0:3],
                                axis=mybir.AxisListType.X,
                                op=mybir.AluOpType.add)
        fps = psum.tile([126, 2048], F32, tag="ps", name=f"fold_ps_{qb}")
        nc.tensor.matmul(fps[0:qs, 0:1], foldc[qs][0:3 * qs, 0:qs],
                         csum[0:3 * qs, 0:1], start=True, stop=True)
        rec = small.tile([qs, 1], F32, tag="rec", name=f"rec_{qb}")
        nc.vector.reciprocal(rec[0:qs, 0:1], fps[0:qs, 0:1])
        ups = psum.tile([126, 2048], F32, tag="ps", name=f"unf_ps_{qb}")
        nc.tensor.matmul(ups[0:3 * qs, 0:1], unfoldc[qs][0:qs, 0:3 * qs],
                         rec[0:qs, 0:1], start=True, stop=True)
        rp = small.tile([126, 1], F32, tag="recP", bufs=2,
                        name=f"recP_{qb}")
        nc.vector.tensor_copy(rp[0:3 * qs, 0:1], ups[0:3 * qs, 0:1])

        # normalize per head-row and write out in column halves so the
        # flush reaches both DMA queues immediately; the last block also
        # uses the scalar queue (no exp work left to delay).
        engines = [nc.sync, nc.gpsimd] + ([nc.scalar] if final else [])
        half = S // 2
        for g, (hg, h0) in enumerate(HGROUPS):
            mp = hg * qs
            eb = expb[(qb, g)]
            # whole-tile: DVE start partitions must be 32-aligned
            nc.vector.tensor_scalar_mul(
                eb[0:mp, 0:S], eb[0:mp, 0:S], rp[0:mp, 0:1],
            )
            for rr in range(hg):
                r0 = rr * qs
                for c0, cw in ((0, half), (half, S - half)):
                    eng = engines[dma_idx % len(engines)]
                    dma_idx += 1
                    eng.dma_start(
                        out=out_r[h0 + rr:h0 + rr + 1, q0:q0 + qs,
                                  c0:c0 + cw],
                        in_=eb[r0:r0 + qs, c0:c0 + cw],
                    )

    # software pipeline:
    #  - kproj chunks interleave 1:1 with block 0 / group 0 score chunks
    #    (each score chunk only needs the kp columns the kproj chunk
    #    just produced), so the first exp lands ~15us in instead of
    #    waiting for the whole kproj phase.
    #  - fin(i) sits between scores(i+1, g0) and scores(i+1, g1) in PE
    #    program order — by the time the PE has pushed block i+1's
    #    group-0 matmuls (paced by the exp stream's PSUM rotation),
    #    block i's sums are ready, so the fold chain runs without
    #    stalling the PE and the flush starts as early as possible.
    for qb in range(len(QBLOCKS)):
        emit_sums_tile(qb)
    eb00, parts00 = alloc_scores(0, 0)
    nchunks = _chunks(S, 2048)
    for ci, (c0, cw) in enumerate(nchunks):
        kproj_chunk(ci, c0, cw)
        emit_score_chunk(0, 0, ci, c0, cw, eb00, parts00)
    emit_score_reduce(0, 0, parts00, len(nchunks))
    emit_scores(0, [1, 2])
    emit_scores(1, [0])
    emit_finish(0)
    emit_scores(1, [1, 2])
    emit_scores(2, [0])
    emit_finish(1)
    emit_scores(2, [1, 2])
    emit_scores(3, [0])
    emit_finish(2)
    emit_scores(3, [1, 2])
    emit_finish(3, final=True)


_CACHED = {}


def _build(use_qbias, use_kbias):
    key = (use_qbias, use_kbias)
    if key in _CACHED:
        return _CACHED[key]
    nc = bacc.Bacc("TRN2", target_bir_lowering=False, debug=False)
    d = {}
    d["qT"] = nc.dram_tensor("qT", [256, QS], F16, kind="ExternalInput")
    d["k"] = nc.dram_tensor("k", [256, S], F16, kind="ExternalInput")
    d["maskb"] = nc.dram_tensor("maskb", [1, S], F16, kind="ExternalInput")
    d["q_wT"] = nc.dram_tensor("q_wT", [256, 256], F16, kind="ExternalInput")
    d["k_wT"] = nc.dram_tensor("k_wT", [256, 256], F16, kind="ExternalInput")
    for qs in QSIZES:
        d[f"fold{qs}"] = nc.dram_tensor(f"fold{qs}", [3 * qs, qs], F32,
                                        kind="ExternalInput")
        d[f"unfold{qs}"] = nc.dram_tensor(f"unfold{qs}", [qs, 3 * qs], F32,
                                          kind="ExternalInput")
    if use_qbias:
        for g, (hg, h0) in enumerate(HGROUPS):
            d[f"qbias{g}"] = nc.dram_tensor(f"qbias{g}", [32 * hg, 1], F32,
                                            kind="ExternalInput")
    if use_kbias:
        for g, (hg, h0) in enumerate(HGROUPS):
            d[f"kbias{g}"] = nc.dram_tensor(f"kbias{g}", [32 * hg, 1], F32,
                                            kind="ExternalInput")
    d["out"] = nc.dram_tensor("out", [QS, NH * S], OUT_DTYPE,
                              kind="ExternalOutput")
    from contextlib import ExitStack
    with tile.TileContext(nc) as tc:
        with ExitStack() as ctx:
            _emit(nc, tc, ctx, d, use_qbias, use_kbias)
    nc.compile()
    _CACHED[key] = nc
    return nc


def make_in_maps(q, k, mask, q_w, q_b, k_w, k_b):
    use_qbias = bool(np.any(q_b != 0))
    use_kbias = bool(np.any(k_b != 0))
    shared = {
        "q_wT": np.ascontiguousarray(q_w.T).astype(np.float16),
        "k_wT": np.ascontiguousarray(k_w.T).astype(np.float16),
    }
    shared.update(_fold_consts())
    if use_qbias:
        qb_scaled = (q_b.astype(np.float32) * NORM_FACT).reshape(256, 1)
        for g, (hg, h0) in enumerate(HGROUPS):
            shared[f"qbias{g}"] = np.ascontiguousarray(
                qb_scaled[32 * h0:32 * h0 + 32 * hg])
    if use_kbias:
        kb_col = k_b.astype(np.float32).reshape(256, 1)
        for g, (hg, h0) in enumerate(HGROUPS):
            shared[f"kbias{g}"] = np.ascontiguousarray(
                kb_col[32 * h0:32 * h0 + 32 * hg])

    in_maps = []
    for c in range(NCORES):
        b, qh = c // 2, c % 2
        m = dict(shared)
        m["qT"] = np.ascontiguousarray(
            q[b, qh * QS:(qh + 1) * QS, :].T).astype(np.float16)
        m["k"] = k[b].reshape(256, S).astype(np.float16)
        m["maskb"] = np.where(mask[b].reshape(1, S), np.float16(MASK_NEG),
                              np.float16(0.0))
        in_maps.append(m)
    return in_maps, use_qbias, use_kbias


def assemble(outs):
    """outs: list of 8 per-core [QS, NH*S] arrays -> [B, Q, NH, H, W] f32."""
    full = np.empty((B, Q, NH, H, W), np.float32)
    for c in range(NCORES):
        b, qh = c // 2, c % 2
        # single-pass cast-assign into the contiguous destination view
        full[b, qh * QS:(qh + 1) * QS].reshape(QS, NH * S)[...] = outs[c]
    return full


def kernel(q, k, mask, q_w, q_b, k_w, k_b, _trace=False):
    in_maps, use_qbias, use_kbias = make_in_maps(q, k, mask, q_w, q_b, k_w, k_b)
    nc = _build(use_qbias, use_kbias)
    res = run_bass_kernel_spmd(nc, in_maps, core_ids=list(range(NCORES)),
                               trace=_trace)
    out = assemble([r["out"] for r in res.results])
    if _trace:
        return out, res
    return out


# revision 4
# speedup vs baseline: 42735.7208x; 1.0186x over previous
# Trainium2 Bass kernel for nn_MHAttentionMap (DETR-style attention map).
#
# Reference computation:
#   qp = q @ q_w.T + q_b                       [b, Q, 256]
#   kp = 1x1conv(k, k_w) + k_b                 [b, 256, H, W]
#   scores[b,q,n,s] = (qh*NORM) . kh           [b, Q, 8, H*W]
#   scores[mask] = -inf ; softmax over flattened (n, H, W) per (b, q)
#
# Sharding: 8 cores = (batch 0..3) x (query half 0..1); 150 queries/core.
# The softmax axis (heads x spatial) lives entirely on one core, so no
# collectives are needed.
#
# Optimizations vs the 570us baseline (measured ~222-226us device time):
#   - q/k/weights shipped fp16: kproj matmuls at full PE rate (fp32 is
#     quarter-rate and made kproj a 125us serial phase); k input DMA
#     halves to 5.1MB/core.
#   - 4 streaming query blocks (42/42/42/24) with double-buffered exp
#     tiles instead of 2 serialized rounds.
#   - fin(i) (fold chain + normalize + out DMA) is emitted between
#     scores(i+1, g0) and scores(i+1, g1): emitted after ALL of
#     scores(i+1), the fold matmul sits behind matmuls that are paced by
#     PSUM rotation against the exp stream, so every block's flush
#     started ~20us late and starved the exp stream of its (bufs=2)
#     buffers. ACT is the critical engine (~122us exp stream, column-
#     paced); this ordering keeps it saturated.
#   - kproj chunks interleave 1:1 with block-0 score chunks so the
#     first exp lands early instead of after the whole kproj phase.
#   - k input DMAs alternate sync/gpsimd (both idle early); the scalar
#     queue carries no DMAs until the final flush so exp is never
#     blocked behind a transfer.
#   - column-split output DMAs round-robin sync/gpsimd + final block
#     flushing over three queues (sync/gpsimd/scalar) to shrink the
#     drain tail.
#   - qproj scale on DVE, not ACT.
#
# Per-core device program (identical on all cores, different data):
#   - qproj on PE -> qpT group tiles (fp16), NORM_FACT folded in
#   - kproj on PE (K=256, fp16) -> kp tiles [97/97/65, 10000] fp16 grouped
#     as heads (0-2), (3-5), (6-7); last row of each = mask bias row
#     (0 / -30000, fp16, marshaled on host)
#   - scores: block-diagonal matmuls packing (heads-in-group x query-block)
#     into M<=126 with a ones-row in lhsT so the mask bias adds inside the
#     matmul (K = 32*hg + 1)
#   - exp on ACT directly from PSUM into fp16 block buffers with
#     accum_out partial row sums; fold/unfold 0/1 matmuls (host constants)
#     reduce per-(head,query) sums into per-query totals and broadcast the
#     reciprocal back to the block layout
#   - in-place DVE normalize, per-head DMA out on alternating queues

import numpy as np

import concourse.bacc as bacc
import concourse.bass as bass
import concourse.mybir as mybir
import concourse.tile as tile
from concourse.bass_utils import run_bass_kernel_spmd

QUERY_DIM = 256
HIDDEN = 256
NH = 8
HD = HIDDEN // NH  # 32
NORM_FACT = float(HIDDEN / NH) ** (-0.5)

B = 4
Q = 300
H = 100
W = 100
S = H * W  # 10000
NCORES = 8
QS = Q // 2  # 150 queries per core

# head groups: (#heads, first head)
HGROUPS = [(3, 0), (3, 3), (2, 6)]
# query blocks: (offset, size)
QBLOCKS = [(0, 42), (42, 42), (84, 42), (126, 24)]
QSIZES = (42, 24)

MASK_NEG = -30000.0

# fp16 output halves the dominant HBM write; verified <2e-3 rel err.
OUT_DTYPE = mybir.dt.float16

F32 = mybir.dt.float32
F16 = mybir.dt.float16


def _chunks(total, size):
    out = []
    off = 0
    while off < total:
        out.append((off, min(size, total - off)))
        off += size
    return out


_FOLD_CONSTS = None


def _fold_consts():
    # fold[qs*r + j, j] = 1 folds 3 stacked per-head rows into per-query;
    # unfold is its transpose (broadcast back to block layout).
    global _FOLD_CONSTS
    if _FOLD_CONSTS is None:
        consts = {}
        for qs in QSIZES:
            fold = np.zeros((3 * qs, qs), np.float32)
            for r in range(3):
                fold[qs * r + np.arange(qs), np.arange(qs)] = 1.0
            consts[f"fold{qs}"] = fold
            consts[f"unfold{qs}"] = np.ascontiguousarray(fold.T)
        _FOLD_CONSTS = consts
    return _FOLD_CONSTS


def _emit(nc, tc, ctx, d, use_qbias, use_kbias):
    """Emit the per-core program. d: dict of DRAM tensor handles."""
    consts = ctx.enter_context(tc.tile_pool(name="consts", bufs=1))
    persist = ctx.enter_context(tc.tile_pool(name="persist", bufs=1))
    work = ctx.enter_context(tc.tile_pool(name="work", bufs=3))
    small = ctx.enter_context(tc.tile_pool(name="small", bufs=4))
    psum = ctx.enter_context(tc.tile_pool(name="psum", bufs=2, space="PSUM"))

    # ---- load constants (sync queue; small) ----
    # qproj inputs first so the PE can start immediately
    qwT = []
    qT = []
    for kb in range(2):
        t = consts.tile([128, 256], F16, tag=f"qwT{kb}", name=f"qwT{kb}")
        nc.sync.dma_start(out=t, in_=d["q_wT"][kb * 128:(kb + 1) * 128, :])
        qwT.append(t)
        t2 = consts.tile([128, QS], F16, tag=f"qT{kb}", name=f"qT{kb}")
        nc.sync.dma_start(out=t2, in_=d["qT"][kb * 128:(kb + 1) * 128, :])
        qT.append(t2)
    kwT = []
    for kb in range(2):
        t = consts.tile([128, 256], F16, tag=f"kwT{kb}", name=f"kwT{kb}")
        nc.sync.dma_start(out=t, in_=d["k_wT"][kb * 128:(kb + 1) * 128, :])
        kwT.append(t)
    foldc = {}
    unfoldc = {}
    for qs in QSIZES:
        f = consts.tile([3 * qs, qs], F32, tag=f"fold{qs}", name=f"fold{qs}")
        nc.sync.dma_start(out=f, in_=d[f"fold{qs}"][:, :])
        foldc[qs] = f
        u = consts.tile([qs, 3 * qs], F32, tag=f"unfold{qs}", name=f"unfold{qs}")
        nc.sync.dma_start(out=u, in_=d[f"unfold{qs}"][:, :])
        unfoldc[qs] = u
    qbias_t = []
    kbias_t = []
    if use_qbias or use_kbias:
        for g, (hg, h0) in enumerate(HGROUPS):
            mg = 32 * hg
            if use_qbias:
                t = consts.tile([mg, 1], F32, tag=f"qb{g}", name=f"qb{g}")
                nc.sync.dma_start(out=t, in_=d[f"qbias{g}"][:, :])
                qbias_t.append(t)
            if use_kbias:
                t = consts.tile([mg, 1], F32, tag=f"kb{g}", name=f"kb{g}")
                nc.sync.dma_start(out=t, in_=d[f"kbias{g}"][:, :])
                kbias_t.append(t)

    # ---- qproj: qpT_g[g] [32*hg, 150] fp16 = (q_w @ q.T + q_b) * NORM ----
    # scale on DVE, not ACT: the ACT engine is the critical exp stream.
    qpT = []
    for g, (hg, h0) in enumerate(HGROUPS):
        mg = 32 * hg
        moff = 32 * h0
        ps = psum.tile([126, 2048], F32, tag="ps", name=f"qproj_ps{g}")
        for kb in range(2):
            nc.tensor.matmul(
                ps[0:mg, 0:QS],
                qwT[kb][:, moff:moff + mg],
                qT[kb][:, 0:QS],
                start=(kb == 0),
                stop=(kb == 1),
            )
        t = persist.tile([mg, QS], F16, tag=f"qpT{g}", name=f"qpT{g}")
        nc.vector.tensor_scalar_mul(t[0:mg, 0:QS], ps[0:mg, 0:QS], NORM_FACT)
        if use_qbias:
            # host ships q_b * NORM_FACT, so add after the scale
            nc.vector.tensor_scalar_add(
                t[0:mg, 0:QS], t[0:mg, 0:QS], qbias_t[g][0:mg, 0:1])
        qpT.append(t)

    # ---- block-diagonal lhsT staging tiles (all query blocks) ----
    # stg[(qb, g)]: [K_g, M_p] fp16, K_g = 32*hg + 1 (ones row last),
    # block r at rows 32r..32r+32, cols r*qs..(r+1)*qs.
    stg = {}
    for qb, (q0, qs) in enumerate(QBLOCKS):
        for g, (hg, h0) in enumerate(HGROUPS):
            kg = 32 * hg + 1
            mp = hg * qs
            t = persist.tile([kg, 126], F16, tag=f"stg_{qb}_{g}",
                             name=f"stg_{qb}_{g}")
            nc.vector.memset(t, 0.0)
            for rr in range(hg):
                nc.vector.tensor_copy(
                    t[32 * rr:32 * rr + 32, rr * qs:(rr + 1) * qs],
                    qpT[g][32 * rr:32 * rr + 32, q0:q0 + qs],
                )
            nc.vector.memset(t[kg - 1:kg, 0:mp], 1.0)
            stg[(qb, g)] = t

    # ---- kproj: kp[g] [32*hg + 1, S] fp16, mask row last ----
    kp = []
    for g, (hg, h0) in enumerate(HGROUPS):
        kg = 32 * hg + 1
        t = persist.tile([kg, S], F16, tag=f"kp{g}", name=f"kp{g}")
        nc.sync.dma_start(out=t[kg - 1:kg, :], in_=d["maskb"][0:1, :])
        kp.append(t)

    # k input chunks stream through small double-buffered tiles on the
    # sync+gpsimd queues (both idle early; scalar stays clean for exp).
    def kproj_chunk(ci, c0, cw):
        kin = []
        for kb in range(2):
            t = work.tile([128, 2048], F16, tag=f"kin{kb}", bufs=2,
                          name=f"kin{kb}_{c0}")
            eng = nc.sync if (ci + kb) % 2 == 0 else nc.gpsimd
            eng.dma_start(out=t[:, 0:cw],
                          in_=d["k"][kb * 128:(kb + 1) * 128, c0:c0 + cw])
            kin.append(t)
        for g, (hg, h0) in enumerate(HGROUPS):
            mg = 32 * hg
            moff = 32 * h0
            ps = psum.tile([126, 2048], F32, tag="ps",
                           name=f"kproj_ps{g}_{c0}")
            for js, nw in _chunks(cw, 512):
                for kb in range(2):
                    nc.tensor.matmul(
                        ps[0:mg, js:js + nw],
                        kwT[kb][:, moff:moff + mg],
                        kin[kb][:, js:js + nw],
                        start=(kb == 0),
                        stop=(kb == 1),
                    )
            if use_kbias:
                nc.vector.tensor_scalar_add(
                    kp[g][0:mg, c0:c0 + cw], ps[0:mg, 0:cw],
                    kbias_t[g][0:mg, 0:1],
                )
            else:
                nc.vector.tensor_copy(kp[g][0:mg, c0:c0 + cw],
                                      ps[0:mg, 0:cw])

    # ---- streaming query blocks: scores -> exp(+accum) -> (deferred
    # fold -> normalize -> out) so block i+1 compute overlaps block i
    # normalize + DMA out. expb rotates 2 buffers per group.
    sums = {}
    expb = {}
    out_r = d["out"][:].rearrange("q (h s) -> h q s", h=NH)
    dma_idx = 0

    def emit_sums_tile(qb):
        t = small.tile([126, 3], F32, tag="sums", bufs=4, name=f"sums_{qb}")
        nc.vector.memset(t, 0.0)
        sums[qb] = t

    def alloc_scores(qb, g):
        eb = work.tile([126, S], F16, tag=f"expb_{g}", bufs=2,
                       name=f"expb_{qb}_{g}")
        expb[(qb, g)] = eb
        parts = small.tile([126, 5], F32, tag="parts", bufs=3,
                           name=f"parts_{qb}_{g}")
        return eb, parts

    def emit_score_chunk(qb, g, ci, c0, cw, eb, parts):
        q0, qs = QBLOCKS[qb]
        hg, h0 = HGROUPS[g]
        kg = 32 * hg + 1
        mp = hg * qs
        lhs = stg[(qb, g)]
        ps = psum.tile([126, 2048], F32, tag="ps",
                       name=f"sc_ps_{qb}_{g}_{c0}")
        for js, nw in _chunks(cw, 512):
            nc.tensor.matmul(
                ps[0:mp, js:js + nw],
                lhs[0:kg, 0:mp],
                kp[g][0:kg, c0 + js:c0 + js + nw],
                start=True, stop=True,
            )
        nc.scalar.activation(
            eb[0:mp, c0:c0 + cw], ps[0:mp, 0:cw],
            mybir.ActivationFunctionType.Exp,
            accum_out=parts[0:mp, ci:ci + 1],
        )

    def emit_score_reduce(qb, g, parts, nchunks):
        q0, qs = QBLOCKS[qb]
        hg, h0 = HGROUPS[g]
        mp = hg * qs
        nc.vector.tensor_reduce(
            sums[qb][0:mp, g:g + 1], parts[0:mp, 0:nchunks],
            axis=mybir.AxisListType.X, op=mybir.AluOpType.add,
        )

    def emit_scores(qb, groups):
        for g in groups:
            eb, parts = alloc_scores(qb, g)
            nchunks = _chunks(S, 2048)
            for ci, (c0, cw) in enumerate(nchunks):
                emit_score_chunk(qb, g, ci, c0, cw, eb, parts)
            emit_score_reduce(qb, g, parts, len(nchunks))

    def emit_finish(qb, final=False):
        nonlocal dma_idx
        q0, qs = QBLOCKS[qb]
        # per-query totals -> reciprocal -> broadcast to block layout.
        # column-sum first so the PE fold feeds straight into reciprocal.
        csum = small.tile([126, 1], F32, tag="csum", bufs=2,
                          name=f"csum_{qb}")
        nc.vector.tensor_reduce(csum[0:3 * qs, 0:1], sums[qb][0:3 * qs, 0:3],
                                axis=mybir.AxisListType.X,
                                op=mybir.AluOpType.add)
        fps = psum.tile([126, 2048], F32, tag="ps", name=f"fold_ps_{qb}")
        nc.tensor.matmul(fps[0:qs, 0:1], foldc[qs][0:3 * qs, 0:qs],
                         csum[0:3 * qs, 0:1], start=True, stop=True)
        rec = small.tile([qs, 1], F32, tag="rec", name=f"rec_{qb}")
        nc.vector.reciprocal(rec[0:qs, 0:1], fps[0:qs, 0:1])
        ups = psum.tile([126, 2048], F32, tag="ps", name=f"unf_ps_{qb}")
        nc.tensor.matmul(ups[0:3 * qs, 0:1], unfoldc[qs][0:qs, 0:3 * qs],
                         rec[0:qs, 0:1], start=True, stop=True)
        rp = small.tile([126, 1], F32, tag="recP", bufs=2,
                        name=f"recP_{qb}")
        nc.vector.tensor_copy(rp[0:3 * qs, 0:1], ups[0:3 * qs, 0:1])

        # normalize per head-row and write out in column halves so the
        # flush reaches both DMA queues immediately; the last block also
        # uses the scalar queue (no exp work left to delay).
        engines = [nc.sync, nc.gpsimd] + ([nc.scalar] if final else [])
        half = S // 2
        for g, (hg, h0) in enumerate(HGROUPS):
            mp = hg * qs
            eb = expb[(qb, g)]
            # whole-tile: DVE start partitions must be 32-aligned
            nc.vector.tensor_scalar_mul(
                eb[0:mp, 0:S], eb[0:mp, 0:S], rp[0:mp, 0:1],
            )
            for rr in range(hg):
                r0 = rr * qs
                for c0, cw in ((0, half), (half, S - half)):
                    eng = engines[dma_idx % len(engines)]
                    dma_idx += 1
                    eng.dma_start(
                        out=out_r[h0 + rr:h0 + rr + 1, q0:q0 + qs,
                                  c0:c0 + cw],
                        in_=eb[r0:r0 + qs, c0:c0 + cw],
                    )

    # software pipeline:
    #  - kproj chunks interleave 1:1 with block 0 / group 0 score chunks
    #    (each score chunk only needs the kp columns the kproj chunk
    #    just produced), so the first exp lands ~15us in instead of
    #    waiting for the whole kproj phase.
    #  - fin(i) sits between scores(i+1, g0) and scores(i+1, g1) in PE
    #    program order — by the time the PE has pushed block i+1's
    #    group-0 matmuls (paced by the exp stream's PSUM rotation),
    #    block i's sums are ready, so the fold chain runs without
    #    stalling the PE and the flush starts as early as possible.
    for qb in range(len(QBLOCKS)):
        emit_sums_tile(qb)
    eb00, parts00 = alloc_scores(0, 0)
    nchunks = _chunks(S, 2048)
    for ci, (c0, cw) in enumerate(nchunks):
        kproj_chunk(ci, c0, cw)
        emit_score_chunk(0, 0, ci, c0, cw, eb00, parts00)
    emit_score_reduce(0, 0, parts00, len(nchunks))
    emit_scores(0, [1, 2])
    emit_scores(1, [0])
    emit_finish(0)
    emit_scores(1, [1, 2])
    emit_scores(2, [0])
    emit_finish(1)
    emit_scores(2, [1, 2])
    emit_scores(3, [0])
    emit_finish(2)
    emit_scores(3, [1, 2])
    emit_finish(3, final=True)


_CACHED = {}


def _build(use_qbias, use_kbias):
    key = (use_qbias, use_kbias)
    if key in _CACHED:
        return _CACHED[key]
    nc = bacc.Bacc("TRN2", target_bir_lowering=False, debug=False)
    d = {}
    d["qT"] = nc.dram_tensor("qT", [256, QS], F16, kind="ExternalInput")
    d["k"] = nc.dram_tensor("k", [256, S], F16, kind="ExternalInput")
    d["maskb"] = nc.dram_tensor("maskb", [1, S], F16, kind="ExternalInput")
    d["q_wT"] = nc.dram_tensor("q_wT", [256, 256], F16, kind="ExternalInput")
    d["k_wT"] = nc.dram_tensor("k_wT", [256, 256], F16, kind="ExternalInput")
    for qs in QSIZES:
        d[f"fold{qs}"] = nc.dram_tensor(f"fold{qs}", [3 * qs, qs], F32,
                                        kind="ExternalInput")
        d[f"unfold{qs}"] = nc.dram_tensor(f"unfold{qs}", [qs, 3 * qs], F32,
                                          kind="ExternalInput")
    if use_qbias:
        for g, (hg, h0) in enumerate(HGROUPS):
            d[f"qbias{g}"] = nc.dram_tensor(f"qbias{g}", [32 * hg, 1], F32,
                                            kind="ExternalInput")
    if use_kbias:
        for g, (hg, h0) in enumerate(HGROUPS):
            d[f"kbias{g}"] = nc.dram_tensor(f"kbias{g}", [32 * hg, 1], F32,
                                            kind="ExternalInput")
    d["out"] = nc.dram_tensor("out", [QS, NH * S], OUT_DTYPE,
                              kind="ExternalOutput")
    from contextlib import ExitStack
    with tile.TileContext(nc) as tc:
        with ExitStack() as ctx:
            _emit(nc, tc, ctx, d, use_qbias, use_kbias)
    nc.compile()
    _CACHED[key] = nc
    return nc


def make_in_maps(q, k, mask, q_w, q_b, k_w, k_b):
    use_qbias = bool(np.any(q_b != 0))
    use_kbias = bool(np.any(k_b != 0))
    shared = {
        "q_wT": np.ascontiguousarray(q_w.T).astype(np.float16),
        "k_wT": np.ascontiguousarray(k_w.T).astype(np.float16),
    }
    shared.update(_fold_consts())
    if use_qbias:
        qb_scaled = (q_b.astype(np.float32) * NORM_FACT).reshape(256, 1)
        for g, (hg, h0) in enumerate(HGROUPS):
            shared[f"qbias{g}"] = np.ascontiguousarray(
                qb_scaled[32 * h0:32 * h0 + 32 * hg])
    if use_kbias:
        kb_col = k_b.astype(np.float32).reshape(256, 1)
        for g, (hg, h0) in enumerate(HGROUPS):
            shared[f"kbias{g}"] = np.ascontiguousarray(
                kb_col[32 * h0:32 * h0 + 32 * hg])

    in_maps = []
    for c in range(NCORES):
        b, qh = c // 2, c % 2
        m = dict(shared)
        m["qT"] = np.ascontiguousarray(
            q[b, qh * QS:(qh + 1) * QS, :].T).astype(np.float16)
        m["k"] = k[b].reshape(256, S).astype(np.float16)
        m["maskb"] = np.where(mask[b].reshape(1, S), np.float16(MASK_NEG),
                              np.float16(0.0))
        in_maps.append(m)
    return in_maps, use_qbias, use_kbias


def assemble(outs):
    """outs: list of 8 per-core [QS, NH*S] arrays -> [B, Q, NH, H, W] f32."""
    full = np.empty((B, Q, NH, H, W), np.float32)
    for c in range(NCORES):
        b, qh = c // 2, c % 2
        # single-pass cast-assign into the contiguous destination view
        full[b, qh * QS:(qh + 1) * QS].reshape(QS, NH * S)[...] = outs[c]
    return full


def kernel(q, k, mask, q_w, q_b, k_w, k_b, _trace=False):
    in_maps, use_qbias, use_kbias = make_in_maps(q, k, mask, q_w, q_b, k_w, k_b)
    nc = _build(use_qbias, use_kbias)
    res = run_bass_kernel_spmd(nc, in_maps, core_ids=list(range(NCORES)),
                               trace=_trace)
    out = assemble([r["out"] for r in res.results])
    if _trace:
        return out, res
    return out
